# revision 17
# baseline (speedup 1.0000x reference)
"""Trainium2 Bass kernel for nn_ConvTran (conv stem + eRPE transformer + GAP).

Sharding: pure data parallel. B=16 split as 2 batches per core across 8 cores.
All parameters replicated; per-core outputs concatenated on host.
"""

import os
import numpy as np
import ml_dtypes

KDBG = bool(os.environ.get("KDBG"))
KDBG_G = int(os.environ.get("KDBG_G", "0"))

# ---- problem constants (hardcoded; kernel.py must be self-contained) ----
B, S, C_IN, E, H, DFF = 16, 1024, 4, 128, 8, 512
C1 = E * 4          # 512
DH = E // H         # 16
EPS = 1e-5
SCALE = float(E) ** -0.5
N_CORES = 8
NB = B // N_CORES   # batches per core = 2
NG = 2              # head groups of 4
SC = S // 128       # 8 s-chunks
JC = S // 128       # 8 j-chunks
F32 = np.float32


class _Pack:
    """Column-packed [128, N] fp32 constant store."""

    def __init__(self):
        self.cols = []
        self.index = {}
        self.n = 0

    def add(self, name, arr2d):
        a = np.zeros((128, arr2d.shape[1]), F32)
        a[:arr2d.shape[0]] = arr2d
        self.index[name] = (self.n, arr2d.shape[1])
        self.cols.append(a)
        self.n += arr2d.shape[1]

    def finalize(self):
        return np.ascontiguousarray(np.concatenate(self.cols, axis=1))


def _host_prep(inp):
    pk = _Pack()
    f = lambda a: np.asarray(a, dtype=F32)

    # conv1: w [C1,1,1,8] -> lhsT [8, C1] (rows 0-7)
    pk.add("w1c", f(inp["conv1_w"][:, 0, 0, :].T))
    sA = f(inp["bn1_g"]) / np.sqrt(f(inp["bn1_v"]) + EPS)
    pk.add("scaleA", sA.reshape(4, 128).T.astype(F32))
    pk.add("biasA", ((f(inp["conv1_b"]) - f(inp["bn1_m"])) * sA
                     + f(inp["bn1_b"])).reshape(4, 128).T.astype(F32))

    # conv2: [128(c1), 16(k=r*4+cc), 128(e)]
    w2 = f(inp["conv2_w"])[:, :, :, 0]                # [E, C1, 4]
    w2cT = np.zeros((128, 16, 128), F32)
    for r in range(4):
        for cc in range(4):
            w2cT[:, r * 4 + cc, :] = w2[:, cc * 128:(cc + 1) * 128, r].T
    pk.add("w2cT", w2cT.reshape(128, 16 * 128))
    sB = f(inp["bn2_g"]) / np.sqrt(f(inp["bn2_v"]) + EPS)
    pk.add("scaleB", sB[:, None].astype(F32))
    pk.add("biasB", ((f(inp["conv2_b"]) - f(inp["bn2_m"])) * sB
                     + f(inp["bn2_b"]))[:, None].astype(F32))

    # tAPE positional encoding, transposed [E, S]
    pos = np.arange(S, dtype=np.float64)[:, None]
    div = np.exp(np.arange(0, E, 2, dtype=np.float64) * (-np.log(10000.0) / E))
    ang = pos * div * (E / S)
    pe = np.zeros((S, E), np.float64)
    pe[:, 0::2] = np.sin(ang)
    pe[:, 1::2] = np.cos(ang)
    pk.add("peT", pe.astype(F32).T)

    # q/k weights, padded head layout [128, g*128 + 32c + dh]
    def pad_qk(w):
        w = f(w)
        wt = np.zeros((128, NG * 128), F32)
        for g in range(NG):
            for c in range(4):
                h = 4 * g + c
                wt[:, g * 128 + 32 * c:g * 128 + 32 * c + DH] = \
                    w[h * DH:(h + 1) * DH, :].T
        return wt
    pk.add("wqT", pad_qk(inp["wq"]))
    pk.add("wkT", pad_qk(inp["wk"]))
    pk.add("wvT", f(inp["wv"]).T)

    pk.add("ffw1T", f(inp["ff_w1"]).T)
    pk.add("ffb1", f(inp["ff_b1"]).reshape(4, 128).T.astype(F32))
    pk.add("ffw2T", f(inp["ff_w2"]).T.reshape(4, 128, 128)
           .transpose(1, 0, 2).reshape(128, 512))
    pk.add("ffb2", f(inp["ff_b2"])[:, None].astype(F32))

    m = np.arange(128)
    pk.add("bcast4", (m[None, :] // 32 == np.arange(4)[:, None]).astype(F32))
    pk.add("ident", np.eye(128, dtype=F32))
    pk.add("ones", np.ones((128, 1), F32))
    pk.add("eps", np.full((128, 1), EPS, F32))

    lnG = np.stack([f(inp["ln_attn_g"]), f(inp["ln1_g"]), f(inp["ln2_g"])])
    lnB = np.stack([f(inp["ln_attn_b"]), f(inp["ln1_b"]), f(inp["ln2_b"])])
    ln_identity = bool(np.allclose(lnG, 1.0) and np.allclose(lnB, 0.0))
    pk.add("lnG", np.broadcast_to(lnG.reshape(1, 3 * 128), (128, 384)).copy())
    pk.add("lnB", np.broadcast_to(lnB.reshape(1, 3 * 128), (128, 384)).copy())

    d = {"cpack": pk.finalize()}

    # rel_bias diagonal store (bf16): T[jj, h, c] = rel[127 + c - jj, h]
    rel = f(inp["rel_bias"])                          # [2047, 8]
    jj = np.arange(128)[:, None]
    cidx = np.arange(1920)[None, :]
    ts = rel[127 + cidx - jj, :]                      # [128, 1920, 8]
    d["tstore"] = np.ascontiguousarray(
        ts.transpose(0, 2, 1).astype(ml_dtypes.bfloat16))  # [128, 8, 1920]
    return d, pk.index, ln_identity


def _build_bass(index, npack, ln_identity):
    import concourse.bass as bass
    import concourse.bacc as bacc
    import concourse.tile as tile
    from concourse.tile import add_dep_helper
    import concourse.mybir as mybir

    dt = mybir.dt
    AF = mybir.ActivationFunctionType
    ALU = mybir.AluOpType

    nc = bacc.Bacc("TRN2")

    xin = nc.dram_tensor("rhs8", [NB, 8, 4 * S], dt.float32,
                         kind="ExternalInput")
    cpk_dr = nc.dram_tensor("cpack", [128, npack], dt.float32,
                            kind="ExternalInput")
    ts_dr = nc.dram_tensor("tstore", [128, H, 1920], dt.bfloat16,
                           kind="ExternalInput")
    yout = nc.dram_tensor("y", [NB, E], dt.float32, kind="ExternalOutput")
    dbg = {}
    if KDBG:
        for nm, shp in [("xsT", [128, S]), ("xpT", [128, S]),
                        ("qT0", [128, S]), ("kT0", [128, S]),
                        ("v", [128, SC * 128]), ("ut00", [128, 512]),
                        ("osb00", [128, 512]), ("bv00", [128, 512]),
                        ("oatt0", [128, 128]), ("att0", [128, 128]),
                        ("ffT", [128, S])]:
            dbg[nm] = nc.dram_tensor("dbg_" + nm, shp, dt.float32,
                                     kind="ExternalOutput")

    with tile.TileContext(nc) as tc:
        import contextlib
        ctx = contextlib.ExitStack()
        with ctx:
            consts = ctx.enter_context(tc.tile_pool(name="consts", bufs=1))
            cpk = consts.tile([128, npack], dt.float32, tag="cpack")
            nc.sync.dma_start(out=cpk, in_=cpk_dr[:])
            ts_sb = consts.tile([128, H, 1920], dt.bfloat16, tag="tstore")
            nc.sync.dma_start(out=ts_sb, in_=ts_dr[:])

            def C(name, rows=128):
                o, w = index[name]
                return cpk[0:rows, o:o + w]

            w1c_sb = C("w1c", rows=8)
            scaleA_sb, biasA_sb = C("scaleA"), C("biasA")
            w2cT_sb = C("w2cT").rearrange("p (k e) -> p k e", k=16)
            scaleB_sb, biasB_sb = C("scaleB"), C("biasB")
            peT_sb = C("peT")
            wqT_sb = C("wqT").rearrange("p (g e) -> p g e", g=NG)
            wkT_sb = C("wkT").rearrange("p (g e) -> p g e", g=NG)
            wvT_sb = C("wvT")
            ffw1T_sb = C("ffw1T")
            ffb1_sb = C("ffb1")
            ffw2T_sb = C("ffw2T").rearrange("p (k e) -> p k e", k=4)
            ffb2_sb = C("ffb2")
            bcast4_sb = C("bcast4", rows=4)
            ident = C("ident")
            ones_sb = C("ones")
            eps_sb = C("eps")
            lnG_sb = C("lnG").rearrange("p (k e) -> p k e", k=3)
            lnB_sb = C("lnB").rearrange("p (k e) -> p k e", k=3)

            pers = ctx.enter_context(tc.tile_pool(name="pers", bufs=1))
            xpT = [pers.tile([128, S], dt.float32, tag=f"xpT{b}",
                             name=f"xpT{b}") for b in range(NB)]
            xsrc = [pers.tile([128, SC, 128], dt.float32, tag=f"xsrc{b}",
                              name=f"xsrc{b}") for b in range(NB)]

            # =========== PHASE C: conv stem ===========
            with tc.tile_pool(name="h1", bufs=1) as h1p, \
                 tc.tile_pool(name="cps", bufs=2, space="PSUM") as cps, \
                 tc.tile_pool(name="cmisc", bufs=2) as cmisc:
                for b in range(NB):
                    rhs8 = cmisc.tile([8, 4 * S], dt.float32, tag="rhs8",
                                      name="rhs8")
                    nc.sync.dma_start(out=rhs8, in_=xin[b])
                    h1 = [[h1p.tile([128, S], dt.float32, tag=f"h1_{cc}_{r}",
                                    name=f"h1_{cc}_{r}")
                           for r in range(4)] for cc in range(4)]
                    for cc in range(4):
                        for n in range(8):
                            r, sh = n // 2, n % 2
                            ps = cps.tile([128, 512], dt.float32, tag="c1ps",
                                          name="c1ps")
                            nc.tensor.matmul(
                                ps, lhsT=w1c_sb[:, cc * 128:(cc + 1) * 128],
                                rhs=rhs8[:, n * 512:(n + 1) * 512],
                                start=True, stop=True)
                            nc.scalar.activation(
                                h1[cc][r][:, sh * 512:(sh + 1) * 512], ps,
                                AF.Gelu, bias=biasA_sb[:, cc:cc + 1],
                                scale=scaleA_sb[:, cc:cc + 1])
                    xsT = cmisc.tile([128, S], dt.float32, tag="xsT",
                                     name="xsT")
                    for sh in range(2):
                        ps = cps.tile([128, 512], dt.float32, tag="c2ps",
                                      name="c2ps")
                        for k in range(16):
                            r, cc = k // 4, k % 4
                            nc.tensor.matmul(
                                ps, lhsT=w2cT_sb[:, k, :],
                                rhs=h1[cc][r][:, sh * 512:(sh + 1) * 512],
                                start=(k == 0), stop=(k == 15))
                        nc.scalar.activation(
                            xsT[:, sh * 512:(sh + 1) * 512], ps, AF.Gelu,
                            bias=biasB_sb, scale=scaleB_sb)
                    if KDBG and b == 0:
                        nc.sync.dma_start(out=dbg["xsT"][:], in_=xsT)
                    nc.vector.tensor_tensor(xpT[b], xsT, peT_sb, op=ALU.add)
                    if KDBG and b == 0:
                        nc.sync.dma_start(out=dbg["xpT"][:], in_=xpT[b])
                    for sc in range(SC):
                        ps = cps.tile([128, 128], dt.float32, tag="tps",
                                      name="tps")
                        nc.tensor.transpose(ps, xsT[:, sc * 128:(sc + 1) * 128],
                                            ident)
                        nc.vector.tensor_copy(out=xsrc[b][:, sc, :], in_=ps)

            # =========== attention + tail pools ===========
            scp = ctx.enter_context(
                tc.tile_pool(name="scp", bufs=1, space="PSUM"))
            pvp = ctx.enter_context(
                tc.tile_pool(name="pvp", bufs=2, space="PSUM"))
            bvp = ctx.enter_context(
                tc.tile_pool(name="bvp", bufs=1, space="PSUM"))
            msp = ctx.enter_context(
                tc.tile_pool(name="msp", bufs=1, space="PSUM"))
            utp = ctx.enter_context(tc.tile_pool(name="utp", bufs=1))
            qkv = ctx.enter_context(tc.tile_pool(name="qkv", bufs=1))
            att_p = ctx.enter_context(tc.tile_pool(name="attp", bufs=1))
            sm = ctx.enter_context(tc.tile_pool(name="sm", bufs=2))

            for b in range(NB):
                # ---------- QKV ----------
                qT, kT = [], []
                for g in range(NG):
                    qt = qkv.tile([128, S], dt.float32, tag=f"qt{g}",
                                  name=f"qt{g}")
                    kt = qkv.tile([128, S], dt.float32, tag=f"kt{g}",
                                  name=f"kt{g}")
                    for sh in range(2):
                        ps = msp.tile([128, 512], dt.float32, tag="ms",
                                      name="msq")
                        nc.tensor.matmul(ps, lhsT=wqT_sb[:, g, :],
                                         rhs=xpT[b][:, sh * 512:(sh + 1) * 512],
                                         start=True, stop=True)
                        nc.vector.tensor_copy(
                            out=qt[:, sh * 512:(sh + 1) * 512], in_=ps)
                        ps2 = msp.tile([128, 512], dt.float32, tag="ms",
                                       name="msk")
                        nc.tensor.matmul(ps2, lhsT=wkT_sb[:, g, :],
                                         rhs=xpT[b][:, sh * 512:(sh + 1) * 512],
                                         start=True, stop=True)
                        nc.vector.tensor_copy(
                            out=kt[:, sh * 512:(sh + 1) * 512], in_=ps2)
                    if KDBG and b == 0 and g == KDBG_G:
                        nc.sync.dma_start(out=dbg["qT0"][:], in_=qt)
                        nc.sync.dma_start(out=dbg["kT0"][:], in_=kt)
                    qT.append(qt)
                    kT.append(kt)
                v_sb = qkv.tile([128, SC, 128], dt.float32, tag="v", name="v")
                for sc in range(SC):
                    ps = msp.tile([128, 128], dt.float32, tag="ms", name="msv")
                    nc.tensor.matmul(ps,
                                     lhsT=xpT[b][:, sc * 128:(sc + 1) * 128],
                                     rhs=wvT_sb, start=True, stop=True)
                    nc.vector.tensor_copy(out=v_sb[:, sc, :], in_=ps)
                if KDBG and b == 0:
                    nc.sync.dma_start(
                        out=dbg["v"][:],
                        in_=v_sb.rearrange("p a b -> p (a b)"))
                VO, VB = [], []
                for g in range(NG):
                    vo = qkv.tile([128, JC, 4, 32], dt.float32, tag=f"vo{g}",
                                  name=f"vo{g}")
                    vb = qkv.tile([128, JC, 4, DH], dt.bfloat16, tag=f"vb{g}",
                                  name=f"vb{g}")
                    vsrc = v_sb.rearrange("p jc (h d) -> p jc h d", h=H)
                    nc.vector.tensor_copy(out=vo[:, :, :, 0:DH],
                                          in_=vsrc[:, :, 4 * g:4 * g + 4, :])
                    nc.vector.memset(vo[:, :, :, DH:DH + 1], 1.0)
                    nc.vector.tensor_copy(out=vb,
                                          in_=vsrc[:, :, 4 * g:4 * g + 4, :])
                    VO.append(vo)
                    VB.append(vb)

                oatt = [att_p.tile([128, 128], dt.float32, tag=f"oatt{sc}",
                                   name=f"oatt{sc}") for sc in range(SC)]
                # ---------- attention ----------
                for g in range(NG):
                    for ih in range(2):
                        i0 = ih * 512
                        pv = pvp.tile([128, 512], dt.float32, tag="pv",
                                      name="pv")
                        bv = bvp.tile([128, 512], dt.float32, tag="bv",
                                      name="bv")
                        for jc in range(JC):
                            for c in range(4):
                                st = scp.tile([128, 512], dt.float32,
                                              tag=f"sc{c}", name=f"sc{c}")
                                nc.tensor.matmul(
                                    st,
                                    lhsT=kT[g][32 * c:32 * c + DH,
                                               jc * 128:(jc + 1) * 128],
                                    rhs=qT[g][32 * c:32 * c + DH, i0:i0 + 512],
                                    start=True, stop=True,
                                    tile_position=(32 * c, 0))
                                ut = utp.tile([128, 512], dt.float32,
                                              tag=f"ut{c}", name=f"ut{c}",
                                              bufs=2)
                                nc.scalar.activation(ut, st, AF.Exp,
                                                     scale=SCALE)
                                if (KDBG and b == 0 and g == KDBG_G and ih == 0
                                        and jc == 0 and c == 0):
                                    nc.sync.dma_start(out=dbg["ut00"][:],
                                                      in_=ut)
                                nc.tensor.matmul(
                                    pv[32 * c:32 * c + DH + 1, :],
                                    lhsT=VO[g][:, jc, c, 0:DH + 1],
                                    rhs=ut,
                                    start=(jc == 0),
                                    stop=(jc == JC - 1),
                                    skip_group_check=True,
                                    tile_position=(0, 32 * c))
                                nc.tensor.matmul(
                                    bv[32 * c:32 * c + DH, :],
                                    lhsT=VB[g][:, jc, c, :],
                                    rhs=ts_sb[:, 4 * g + c,
                                        896 - 128 * jc + i0:
                                        896 - 128 * jc + i0 + 512],
                                    start=(jc == 0),
                                    stop=(jc == JC - 1),
                                    skip_group_check=True,
                                    tile_position=(0, 32 * c))
                        osb = sm.tile([128, 512], dt.float32, tag="osb",
                                      bufs=1, name="osb")
                        nc.vector.tensor_copy(out=osb, in_=pv)
                        rs = sm.tile([4, 512], dt.float32, tag="rs", bufs=1,
                                     name="rs")
                        for c in range(4):
                            nc.sync.dma_start(
                                out=rs[c:c + 1, :],
                                in_=osb[32 * c + DH:32 * c + DH + 1, :])
                        rr = sm.tile([4, 512], dt.float32, tag="rr", bufs=1,
                                     name="rr")
                        nc.vector.reciprocal(out=rr, in_=rs)
                        rsb = msp.tile([128, 512], dt.float32, tag="ms",
                                       name="rsb", bufs=1)
                        nc.tensor.matmul(rsb, lhsT=bcast4_sb, rhs=rr,
                                         start=True, stop=True)
                        nc.vector.tensor_tensor(osb, osb, rsb, op=ALU.mult)
                        if KDBG and b == 0 and g == KDBG_G and ih == 0:
                            bvc = sm.tile([128, 512], dt.float32, tag="bvc",
                                          name="bvc", bufs=1)
                            nc.vector.tensor_copy(out=bvc, in_=bv)
                            nc.sync.dma_start(out=dbg["bv00"][:], in_=bvc)
                        nc.vector.tensor_tensor(osb, osb, bv, op=ALU.add)
                        if KDBG and b == 0 and g == KDBG_G and ih == 0:
                            nc.sync.dma_start(out=dbg["osb00"][:], in_=osb)
                        for ic in range(4):
                            ps = msp.tile([128, 128], dt.float32, tag="ms",
                                          name="mst")
                            nc.tensor.transpose(
                                ps, osb[:, ic * 128:(ic + 1) * 128], ident)
                            sc = ih * 4 + ic
                            psr = ps.rearrange("p (c m) -> p c m", c=4)
                            nc.vector.tensor_copy(
                                out=oatt[sc].rearrange(
                                    "p (h d) -> p h d",
                                    h=H)[:, 4 * g:4 * g + 4, :],
                                in_=psr[:, :, 0:DH])

                # ---------- LN / FFN / GAP ----------
                def layer_norm(dst, src_ap, k):
                    stats = sm.tile([128, 6], dt.float32, tag="stats",
                                    name="stats")
                    nc.vector.bn_stats(out=stats, in_=src_ap)
                    mv = sm.tile([128, 2], dt.float32, tag="mv", name="mv")
                    nc.vector.bn_aggr(out=mv, in_=stats)
                    sd = sm.tile([128, 1], dt.float32, tag="sd", name="sd")
                    nc.scalar.activation(sd, mv[:, 1:2], AF.Sqrt, bias=eps_sb)
                    rstd = sm.tile([128, 1], dt.float32, tag="rstd",
                                   name="rstd")
                    nc.vector.reciprocal(out=rstd, in_=sd)
                    nc.vector.tensor_scalar(dst, src_ap, mv[:, 0:1], rstd,
                                            ALU.subtract, ALU.mult)
                    if not ln_identity:
                        nc.vector.tensor_tensor(dst, dst, lnG_sb[:, k, :],
                                                op=ALU.mult)
                        nc.vector.tensor_tensor(dst, dst, lnB_sb[:, k, :],
                                                op=ALU.add)

                if KDBG and b == 0:
                    nc.sync.dma_start(out=dbg["oatt0"][:], in_=oatt[0])
                att = [att_p.tile([128, 128], dt.float32, tag=f"att{sc}",
                                  name=f"att{sc}") for sc in range(SC)]
                attT = att_p.tile([128, S], dt.float32, tag="attT",
                                  name="attT")
                for sc in range(SC):
                    o1 = sm.tile([128, 128], dt.float32, tag="o1", name="o1")
                    layer_norm(o1, oatt[sc], 0)
                    nc.vector.tensor_tensor(o1, o1, xsrc[b][:, sc, :],
                                            op=ALU.add)
                    layer_norm(att[sc], o1, 1)
                    ps = msp.tile([128, 128], dt.float32, tag="ms",
                                  name="msat")
                    nc.tensor.transpose(ps, att[sc], ident)
                    nc.vector.tensor_copy(
                        out=attT[:, sc * 128:(sc + 1) * 128], in_=ps)
                if KDBG and b == 0:
                    nc.sync.dma_start(out=dbg["att0"][:], in_=att[0])
                hrelu = [att_p.tile([128, S], dt.float32, tag=f"hr{fc}",
                                    name=f"hr{fc}") for fc in range(4)]
                for fc in range(4):
                    for sh in range(2):
                        ps = msp.tile([128, 512], dt.float32, tag="ms",
                                      name="msf1")
                        nc.tensor.matmul(
                            ps, lhsT=ffw1T_sb[:, fc * 128:(fc + 1) * 128],
                            rhs=attT[:, sh * 512:(sh + 1) * 512],
                            start=True, stop=True)
                        nc.scalar.activation(
                            hrelu[fc][:, sh * 512:(sh + 1) * 512], ps, AF.Relu,
                            bias=ffb1_sb[:, fc:fc + 1])
                ffT = att_p.tile([128, S], dt.float32, tag="ffT", name="ffT")
                for sh in range(2):
                    ps = msp.tile([128, 512], dt.float32, tag="ms",
                                  name="msf2")
                    for fc in range(4):
                        nc.tensor.matmul(
                            ps, lhsT=ffw2T_sb[:, fc, :],
                            rhs=hrelu[fc][:, sh * 512:(sh + 1) * 512],
                            start=(fc == 0), stop=(fc == 3))
                    nc.vector.tensor_scalar(
                        ffT[:, sh * 512:(sh + 1) * 512], ps, ffb2_sb,
                        None, ALU.add)
                if KDBG and b == 0:
                    nc.sync.dma_start(out=dbg["ffT"][:], in_=ffT)
                acc = sm.tile([128, 1], dt.float32, tag="acc", name="acc")
                nc.vector.memset(acc, 0.0)
                for sc in range(SC):
                    ps = msp.tile([128, 128], dt.float32, tag="ms",
                                  name="msft")
                    nc.tensor.transpose(ps, ffT[:, sc * 128:(sc + 1) * 128],
                                        ident)
                    l2in = sm.tile([128, 128], dt.float32, tag="l2in",
                                   name="l2in")
                    nc.vector.tensor_tensor(l2in, att[sc], ps, op=ALU.add)
                    l2o = sm.tile([128, 128], dt.float32, tag="l2o",
                                  name="l2o")
                    layer_norm(l2o, l2in, 2)
                    mps = msp.tile([128, 1], dt.float32, tag="ms", name="msm")
                    nc.tensor.matmul(mps, lhsT=l2o, rhs=ones_sb,
                                     start=True, stop=True)
                    nc.vector.tensor_tensor(acc, acc, mps, op=ALU.add)
                ob = sm.tile([128, 1], dt.float32, tag="ob", name="ob")
                nc.scalar.mul(ob, acc, 1.0 / S)
                nc.sync.dma_start(out=yout[b, :, None], in_=ob)

    nc.compile()
    return nc


_CACHE = {}


def kernel(**inputs):
    inputs = {k: np.asarray(v) for k, v in inputs.items()}
    host, index, ln_identity = _host_prep(inputs)
    key = (ln_identity, host["cpack"].shape[1], KDBG, KDBG_G)
    if key not in _CACHE:
        _CACHE[key] = _build_bass(index, host["cpack"].shape[1], ln_identity)
    nc = _CACHE[key]

    from concourse.bass_utils import run_bass_kernel_spmd
    x = np.asarray(inputs["x"], dtype=F32)                 # [B, S, 4]
    xpad = np.zeros((B, S + 7, C_IN), F32)
    xpad[:, 3:S + 3, :] = x
    rhs8 = np.empty((B, 8, C_IN, S), F32)
    for t in range(8):
        rhs8[:, t] = xpad[:, t:t + S, :].transpose(0, 2, 1)
    rhs8 = np.ascontiguousarray(rhs8.reshape(B, 8, C_IN * S))
    in_maps = []
    for core in range(N_CORES):
        m = {"rhs8": np.ascontiguousarray(rhs8[core * NB:(core + 1) * NB])}
        m.update(host)
        in_maps.append(m)
    res = run_bass_kernel_spmd(nc, in_maps, list(range(N_CORES)))
    if KDBG:
        kernel.dbg = res.results[0]
    outs = [res.results[c]["y"] for c in range(N_CORES)]
    return np.concatenate(outs, axis=0).astype(F32)


# revision 19
# speedup vs baseline: 161.4701x; 161.4701x over previous
"""Trainium2 Bass kernel for nn_ConvTran (conv stem + eRPE transformer + GAP).

Sharding: pure data parallel. B=16 split as 2 batches per core across 8 cores.
All parameters replicated; per-core outputs concatenated on host.
"""

import os
import numpy as np
import ml_dtypes

KDBG = bool(os.environ.get("KDBG"))
KDBG_G = int(os.environ.get("KDBG_G", "0"))

# ---- problem constants (hardcoded; kernel.py must be self-contained) ----
B, S, C_IN, E, H, DFF = 16, 1024, 4, 128, 8, 512
C1 = E * 4          # 512
DH = E // H         # 16
EPS = 1e-5
SCALE = float(E) ** -0.5
N_CORES = 8
NB = B // N_CORES   # batches per core = 2
NG = 2              # head groups of 4
SC = S // 128       # 8 s-chunks
JC = S // 128       # 8 j-chunks
F32 = np.float32


class _Pack:
    """Column-packed [128, N] fp32 constant store."""

    def __init__(self):
        self.cols = []
        self.index = {}
        self.n = 0

    def add(self, name, arr2d):
        a = np.zeros((128, arr2d.shape[1]), F32)
        a[:arr2d.shape[0]] = arr2d
        self.index[name] = (self.n, arr2d.shape[1])
        self.cols.append(a)
        self.n += arr2d.shape[1]

    def finalize(self):
        return np.ascontiguousarray(np.concatenate(self.cols, axis=1))


def _host_prep(inp):
    pk = _Pack()
    f = lambda a: np.asarray(a, dtype=F32)

    # conv1: w [C1,1,1,8] -> lhsT [8, C1] (rows 0-7)
    pk.add("w1c", f(inp["conv1_w"][:, 0, 0, :].T))
    sA = f(inp["bn1_g"]) / np.sqrt(f(inp["bn1_v"]) + EPS)
    pk.add("scaleA", sA.reshape(4, 128).T.astype(F32))
    pk.add("biasA", ((f(inp["conv1_b"]) - f(inp["bn1_m"])) * sA
                     + f(inp["bn1_b"])).reshape(4, 128).T.astype(F32))

    # conv2: [128(c1), 16(k=r*4+cc), 128(e)]
    w2 = f(inp["conv2_w"])[:, :, :, 0]                # [E, C1, 4]
    w2cT = np.zeros((128, 16, 128), F32)
    for r in range(4):
        for cc in range(4):
            w2cT[:, r * 4 + cc, :] = w2[:, cc * 128:(cc + 1) * 128, r].T
    pk.add("w2cT", w2cT.reshape(128, 16 * 128))
    sB = f(inp["bn2_g"]) / np.sqrt(f(inp["bn2_v"]) + EPS)
    pk.add("scaleB", sB[:, None].astype(F32))
    pk.add("biasB", ((f(inp["conv2_b"]) - f(inp["bn2_m"])) * sB
                     + f(inp["bn2_b"]))[:, None].astype(F32))

    # tAPE positional encoding, transposed [E, S]
    pos = np.arange(S, dtype=np.float64)[:, None]
    div = np.exp(np.arange(0, E, 2, dtype=np.float64) * (-np.log(10000.0) / E))
    ang = pos * div * (E / S)
    pe = np.zeros((S, E), np.float64)
    pe[:, 0::2] = np.sin(ang)
    pe[:, 1::2] = np.cos(ang)
    pk.add("peT", pe.astype(F32).T)

    # q/k weights, padded head layout [128, g*128 + 32c + dh]
    def pad_qk(w):
        w = f(w)
        wt = np.zeros((128, NG * 128), F32)
        for g in range(NG):
            for c in range(4):
                h = 4 * g + c
                wt[:, g * 128 + 32 * c:g * 128 + 32 * c + DH] = \
                    w[h * DH:(h + 1) * DH, :].T
        return wt
    pk.add("wqT", pad_qk(inp["wq"]))
    pk.add("wkT", pad_qk(inp["wk"]))
    pk.add("wvT", f(inp["wv"]).T)

    pk.add("ffw1T", f(inp["ff_w1"]).T)
    pk.add("ffb1", f(inp["ff_b1"]).reshape(4, 128).T.astype(F32))
    pk.add("ffw2T", f(inp["ff_w2"]).T.reshape(4, 128, 128)
           .transpose(1, 0, 2).reshape(128, 512))
    pk.add("ffb2", f(inp["ff_b2"])[:, None].astype(F32))

    m = np.arange(128)
    pk.add("bcast4", (m[None, :] // 32 == np.arange(4)[:, None]).astype(F32))
    pk.add("ident", np.eye(128, dtype=F32))
    pk.add("ones", np.ones((128, 1), F32))
    pk.add("eps", np.full((128, 1), EPS, F32))

    lnG = np.stack([f(inp["ln_attn_g"]), f(inp["ln1_g"]), f(inp["ln2_g"])])
    lnB = np.stack([f(inp["ln_attn_b"]), f(inp["ln1_b"]), f(inp["ln2_b"])])
    ln_identity = bool(np.allclose(lnG, 1.0) and np.allclose(lnB, 0.0))
    pk.add("lnG", np.broadcast_to(lnG.reshape(1, 3 * 128), (128, 384)).copy())
    pk.add("lnB", np.broadcast_to(lnB.reshape(1, 3 * 128), (128, 384)).copy())

    d = {"cpack": pk.finalize()}

    # rel_bias diagonal store (bf16): T[jj, h, c] = rel[127 + c - jj, h]
    rel = f(inp["rel_bias"])                          # [2047, 8]
    jj = np.arange(128)[:, None]
    cidx = np.arange(1920)[None, :]
    ts = rel[127 + cidx - jj, :]                      # [128, 1920, 8]
    d["tstore"] = np.ascontiguousarray(
        ts.transpose(0, 2, 1).astype(ml_dtypes.bfloat16))  # [128, 8, 1920]
    return d, pk.index, ln_identity


def _build_bass(index, npack, ln_identity):
    import concourse.bass as bass
    import concourse.bacc as bacc
    import concourse.tile as tile
    from concourse.tile import add_dep_helper
    import concourse.mybir as mybir

    dt = mybir.dt
    AF = mybir.ActivationFunctionType
    ALU = mybir.AluOpType

    nc = bacc.Bacc("TRN2")

    xin = nc.dram_tensor("rhs8", [NB, 8, 4 * S], dt.float32,
                         kind="ExternalInput")
    cpk_dr = nc.dram_tensor("cpack", [128, npack], dt.float32,
                            kind="ExternalInput")
    ts_dr = nc.dram_tensor("tstore", [128, H, 1920], dt.bfloat16,
                           kind="ExternalInput")
    yout = nc.dram_tensor("y", [NB, E], dt.float32, kind="ExternalOutput")
    dbg = {}
    if KDBG:
        for nm, shp in [("xsT", [128, S]), ("xpT", [128, S]),
                        ("qT0", [128, S]), ("kT0", [128, S]),
                        ("v", [128, SC * 128]), ("ut00", [128, 512]),
                        ("osb00", [128, 512]), ("bv00", [128, 512]),
                        ("oatt0", [128, 128]), ("att0", [128, 128]),
                        ("ffT", [128, S])]:
            dbg[nm] = nc.dram_tensor("dbg_" + nm, shp, dt.float32,
                                     kind="ExternalOutput")

    with tile.TileContext(nc) as tc:
        import contextlib
        ctx = contextlib.ExitStack()
        with ctx:
            consts = ctx.enter_context(tc.tile_pool(name="consts", bufs=1))
            cpk = consts.tile([128, npack], dt.float32, tag="cpack")
            nc.sync.dma_start(out=cpk, in_=cpk_dr[:])
            ts_sb = consts.tile([128, H, 1920], dt.bfloat16, tag="tstore")
            nc.sync.dma_start(out=ts_sb, in_=ts_dr[:])

            def C(name, rows=128):
                o, w = index[name]
                return cpk[0:rows, o:o + w]

            w1c_sb = C("w1c", rows=8)
            scaleA_sb, biasA_sb = C("scaleA"), C("biasA")
            w2cT_sb = C("w2cT").rearrange("p (k e) -> p k e", k=16)
            scaleB_sb, biasB_sb = C("scaleB"), C("biasB")
            peT_sb = C("peT")
            wqT_sb = C("wqT").rearrange("p (g e) -> p g e", g=NG)
            wkT_sb = C("wkT").rearrange("p (g e) -> p g e", g=NG)
            wvT_sb = C("wvT")
            ffw1T_sb = C("ffw1T")
            ffb1_sb = C("ffb1")
            ffw2T_sb = C("ffw2T").rearrange("p (k e) -> p k e", k=4)
            ffb2_sb = C("ffb2")
            bcast4_sb = C("bcast4", rows=4)
            ident = C("ident")
            ones_sb = C("ones")
            eps_sb = C("eps")
            lnG_sb = C("lnG").rearrange("p (k e) -> p k e", k=3)
            lnB_sb = C("lnB").rearrange("p (k e) -> p k e", k=3)

            pers = ctx.enter_context(tc.tile_pool(name="pers", bufs=1))
            xpT = [pers.tile([128, S], dt.float32, tag=f"xpT{b}",
                             name=f"xpT{b}") for b in range(NB)]
            xsrc = [pers.tile([128, SC, 128], dt.float32, tag=f"xsrc{b}",
                              name=f"xsrc{b}") for b in range(NB)]

            # =========== PHASE C: conv stem ===========
            with tc.tile_pool(name="h1", bufs=1) as h1p, \
                 tc.tile_pool(name="cps", bufs=2, space="PSUM") as cps, \
                 tc.tile_pool(name="cmisc", bufs=2) as cmisc:
                for b in range(NB):
                    rhs8 = cmisc.tile([8, 4 * S], dt.float32, tag="rhs8",
                                      name="rhs8")
                    nc.sync.dma_start(out=rhs8, in_=xin[b])
                    h1 = [[h1p.tile([128, S], dt.float32, tag=f"h1_{cc}_{r}",
                                    name=f"h1_{cc}_{r}")
                           for r in range(4)] for cc in range(4)]
                    for cc in range(4):
                        for n in range(8):
                            r, sh = n // 2, n % 2
                            ps = cps.tile([128, 512], dt.float32, tag="c1ps",
                                          name="c1ps")
                            nc.tensor.matmul(
                                ps, lhsT=w1c_sb[:, cc * 128:(cc + 1) * 128],
                                rhs=rhs8[:, n * 512:(n + 1) * 512],
                                start=True, stop=True)
                            nc.scalar.activation(
                                h1[cc][r][:, sh * 512:(sh + 1) * 512], ps,
                                AF.Gelu, bias=biasA_sb[:, cc:cc + 1],
                                scale=scaleA_sb[:, cc:cc + 1])
                    xsT = cmisc.tile([128, S], dt.float32, tag="xsT",
                                     name="xsT")
                    for sh in range(2):
                        ps = cps.tile([128, 512], dt.float32, tag="c2ps",
                                      name="c2ps")
                        for k in range(16):
                            r, cc = k // 4, k % 4
                            nc.tensor.matmul(
                                ps, lhsT=w2cT_sb[:, k, :],
                                rhs=h1[cc][r][:, sh * 512:(sh + 1) * 512],
                                start=(k == 0), stop=(k == 15))
                        nc.scalar.activation(
                            xsT[:, sh * 512:(sh + 1) * 512], ps, AF.Gelu,
                            bias=biasB_sb, scale=scaleB_sb)
                    if KDBG and b == 0:
                        nc.sync.dma_start(out=dbg["xsT"][:], in_=xsT)
                    nc.vector.tensor_tensor(xpT[b], xsT, peT_sb, op=ALU.add)
                    if KDBG and b == 0:
                        nc.sync.dma_start(out=dbg["xpT"][:], in_=xpT[b])
                    for sc in range(SC):
                        ps = cps.tile([128, 128], dt.float32, tag="tps",
                                      name="tps")
                        nc.tensor.transpose(ps, xsT[:, sc * 128:(sc + 1) * 128],
                                            ident)
                        nc.vector.tensor_copy(out=xsrc[b][:, sc, :], in_=ps)

            # =========== attention + tail pools ===========
            scp = ctx.enter_context(
                tc.tile_pool(name="scp", bufs=1, space="PSUM"))
            pvp = ctx.enter_context(
                tc.tile_pool(name="pvp", bufs=2, space="PSUM"))
            bvp = ctx.enter_context(
                tc.tile_pool(name="bvp", bufs=1, space="PSUM"))
            msp = ctx.enter_context(
                tc.tile_pool(name="msp", bufs=1, space="PSUM"))
            utp = ctx.enter_context(tc.tile_pool(name="utp", bufs=1))
            qkv = ctx.enter_context(tc.tile_pool(name="qkv", bufs=1))
            att_p = ctx.enter_context(tc.tile_pool(name="attp", bufs=1))
            sm = ctx.enter_context(tc.tile_pool(name="sm", bufs=2))

            for b in range(NB):
                # ---------- QKV ----------
                qT, kT = [], []
                for g in range(NG):
                    qt = qkv.tile([128, S], dt.float32, tag=f"qt{g}",
                                  name=f"qt{g}")
                    kt = qkv.tile([128, S], dt.float32, tag=f"kt{g}",
                                  name=f"kt{g}")
                    for sh in range(2):
                        ps = msp.tile([128, 512], dt.float32, tag="ms",
                                      name="msq")
                        nc.tensor.matmul(ps, lhsT=wqT_sb[:, g, :],
                                         rhs=xpT[b][:, sh * 512:(sh + 1) * 512],
                                         start=True, stop=True)
                        nc.vector.tensor_copy(
                            out=qt[:, sh * 512:(sh + 1) * 512], in_=ps)
                        ps2 = msp.tile([128, 512], dt.float32, tag="ms",
                                       name="msk")
                        nc.tensor.matmul(ps2, lhsT=wkT_sb[:, g, :],
                                         rhs=xpT[b][:, sh * 512:(sh + 1) * 512],
                                         start=True, stop=True)
                        nc.vector.tensor_copy(
                            out=kt[:, sh * 512:(sh + 1) * 512], in_=ps2)
                    if KDBG and b == 0 and g == KDBG_G:
                        nc.sync.dma_start(out=dbg["qT0"][:], in_=qt)
                        nc.sync.dma_start(out=dbg["kT0"][:], in_=kt)
                    qT.append(qt)
                    kT.append(kt)
                v_sb = qkv.tile([128, SC, 128], dt.float32, tag="v", name="v")
                for sc in range(SC):
                    ps = msp.tile([128, 128], dt.float32, tag="ms", name="msv")
                    nc.tensor.matmul(ps,
                                     lhsT=xpT[b][:, sc * 128:(sc + 1) * 128],
                                     rhs=wvT_sb, start=True, stop=True)
                    nc.vector.tensor_copy(out=v_sb[:, sc, :], in_=ps)
                if KDBG and b == 0:
                    nc.sync.dma_start(
                        out=dbg["v"][:],
                        in_=v_sb.rearrange("p a b -> p (a b)"))
                VO, VB = [], []
                for g in range(NG):
                    vo = qkv.tile([128, JC, 4, 32], dt.float32, tag=f"vo{g}",
                                  name=f"vo{g}")
                    vb = qkv.tile([128, JC, 4, DH], dt.bfloat16, tag=f"vb{g}",
                                  name=f"vb{g}")
                    vsrc = v_sb.rearrange("p jc (h d) -> p jc h d", h=H)
                    nc.vector.tensor_copy(out=vo[:, :, :, 0:DH],
                                          in_=vsrc[:, :, 4 * g:4 * g + 4, :])
                    nc.vector.memset(vo[:, :, :, DH:DH + 1], 1.0)
                    nc.vector.tensor_copy(out=vb,
                                          in_=vsrc[:, :, 4 * g:4 * g + 4, :])
                    VO.append(vo)
                    VB.append(vb)

                oatt = [att_p.tile([128, 128], dt.float32, tag=f"oatt{sc}",
                                   name=f"oatt{sc}") for sc in range(SC)]
                # ---------- attention ----------
                for g in range(NG):
                    for ih in range(2):
                        i0 = ih * 512
                        pv = pvp.tile([128, 512], dt.float32, tag="pv",
                                      name="pv")
                        bv = bvp.tile([128, 512], dt.float32, tag="bv",
                                      name="bv")
                        for jc in range(JC):
                            for c in range(4):
                                st = scp.tile([128, 512], dt.float32,
                                              tag=f"sc{c}", name=f"sc{c}")
                                nc.tensor.matmul(
                                    st,
                                    lhsT=kT[g][32 * c:32 * c + DH,
                                               jc * 128:(jc + 1) * 128],
                                    rhs=qT[g][32 * c:32 * c + DH, i0:i0 + 512],
                                    start=True, stop=True,
                                    tile_position=(32 * c, 0))
                                ut = utp.tile([128, 512], dt.float32,
                                              tag=f"ut{c}", name=f"ut{c}",
                                              bufs=2)
                                nc.scalar.activation(ut, st, AF.Exp,
                                                     scale=SCALE)
                                if (KDBG and b == 0 and g == KDBG_G and ih == 0
                                        and jc == 0 and c == 0):
                                    nc.sync.dma_start(out=dbg["ut00"][:],
                                                      in_=ut)
                                nc.tensor.matmul(
                                    pv[32 * c:32 * c + DH + 1, :],
                                    lhsT=VO[g][:, jc, c, 0:DH + 1],
                                    rhs=ut,
                                    start=(jc == 0),
                                    stop=(jc == JC - 1),
                                    skip_group_check=True,
                                    tile_position=(0, 32 * c))
                                nc.tensor.matmul(
                                    bv[32 * c:32 * c + DH, :],
                                    lhsT=VB[g][:, jc, c, :],
                                    rhs=ts_sb[:, 4 * g + c,
                                        896 - 128 * jc + i0:
                                        896 - 128 * jc + i0 + 512],
                                    start=(jc == 0),
                                    stop=(jc == JC - 1),
                                    skip_group_check=True,
                                    tile_position=(0, 32 * c))
                        osb = sm.tile([128, 512], dt.float32, tag="osb",
                                      bufs=1, name="osb")
                        nc.vector.tensor_copy(out=osb, in_=pv)
                        rs = sm.tile([4, 512], dt.float32, tag="rs", bufs=1,
                                     name="rs")
                        for c in range(4):
                            nc.sync.dma_start(
                                out=rs[c:c + 1, :],
                                in_=osb[32 * c + DH:32 * c + DH + 1, :])
                        rr = sm.tile([4, 512], dt.float32, tag="rr", bufs=1,
                                     name="rr")
                        nc.vector.reciprocal(out=rr, in_=rs)
                        rsb = msp.tile([128, 512], dt.float32, tag="ms",
                                       name="rsb", bufs=1)
                        nc.tensor.matmul(rsb, lhsT=bcast4_sb, rhs=rr,
                                         start=True, stop=True)
                        nc.vector.tensor_tensor(osb, osb, rsb, op=ALU.mult)
                        if KDBG and b == 0 and g == KDBG_G and ih == 0:
                            bvc = sm.tile([128, 512], dt.float32, tag="bvc",
                                          name="bvc", bufs=1)
                            nc.vector.tensor_copy(out=bvc, in_=bv)
                            nc.sync.dma_start(out=dbg["bv00"][:], in_=bvc)
                        nc.vector.tensor_tensor(osb, osb, bv, op=ALU.add)
                        if KDBG and b == 0 and g == KDBG_G and ih == 0:
                            nc.sync.dma_start(out=dbg["osb00"][:], in_=osb)
                        for ic in range(4):
                            ps = msp.tile([128, 128], dt.float32, tag="ms",
                                          name="mst")
                            nc.tensor.transpose(
                                ps, osb[:, ic * 128:(ic + 1) * 128], ident)
                            sc = ih * 4 + ic
                            psr = ps.rearrange("p (c m) -> p c m", c=4)
                            nc.vector.tensor_copy(
                                out=oatt[sc].rearrange(
                                    "p (h d) -> p h d",
                                    h=H)[:, 4 * g:4 * g + 4, :],
                                in_=psr[:, :, 0:DH])

                # ---------- LN / FFN / GAP ----------
                def layer_norm(dst, src_ap, k):
                    stats = sm.tile([128, 6], dt.float32, tag="stats",
                                    name="stats")
                    nc.vector.bn_stats(out=stats, in_=src_ap)
                    mv = sm.tile([128, 2], dt.float32, tag="mv", name="mv")
                    nc.vector.bn_aggr(out=mv, in_=stats)
                    sd = sm.tile([128, 1], dt.float32, tag="sd", name="sd")
                    nc.scalar.activation(sd, mv[:, 1:2], AF.Sqrt, bias=eps_sb)
                    rstd = sm.tile([128, 1], dt.float32, tag="rstd",
                                   name="rstd")
                    nc.vector.reciprocal(out=rstd, in_=sd)
                    nc.vector.tensor_scalar(dst, src_ap, mv[:, 0:1], rstd,
                                            ALU.subtract, ALU.mult)
                    if not ln_identity:
                        nc.vector.tensor_tensor(dst, dst, lnG_sb[:, k, :],
                                                op=ALU.mult)
                        nc.vector.tensor_tensor(dst, dst, lnB_sb[:, k, :],
                                                op=ALU.add)

                if KDBG and b == 0:
                    nc.sync.dma_start(out=dbg["oatt0"][:], in_=oatt[0])
                att = [att_p.tile([128, 128], dt.float32, tag=f"att{sc}",
                                  name=f"att{sc}") for sc in range(SC)]
                attT = att_p.tile([128, S], dt.float32, tag="attT",
                                  name="attT")
                for sc in range(SC):
                    o1 = sm.tile([128, 128], dt.float32, tag="o1", name="o1")
                    layer_norm(o1, oatt[sc], 0)
                    nc.vector.tensor_tensor(o1, o1, xsrc[b][:, sc, :],
                                            op=ALU.add)
                    layer_norm(att[sc], o1, 1)
                    ps = msp.tile([128, 128], dt.float32, tag="ms",
                                  name="msat")
                    nc.tensor.transpose(ps, att[sc], ident)
                    nc.vector.tensor_copy(
                        out=attT[:, sc * 128:(sc + 1) * 128], in_=ps)
                if KDBG and b == 0:
                    nc.sync.dma_start(out=dbg["att0"][:], in_=att[0])
                hrelu = [att_p.tile([128, S], dt.float32, tag=f"hr{fc}",
                                    name=f"hr{fc}") for fc in range(4)]
                for fc in range(4):
                    for sh in range(2):
                        ps = msp.tile([128, 512], dt.float32, tag="ms",
                                      name="msf1")
                        nc.tensor.matmul(
                            ps, lhsT=ffw1T_sb[:, fc * 128:(fc + 1) * 128],
                            rhs=attT[:, sh * 512:(sh + 1) * 512],
                            start=True, stop=True)
                        nc.scalar.activation(
                            hrelu[fc][:, sh * 512:(sh + 1) * 512], ps, AF.Relu,
                            bias=ffb1_sb[:, fc:fc + 1])
                ffT = att_p.tile([128, S], dt.float32, tag="ffT", name="ffT")
                for sh in range(2):
                    ps = msp.tile([128, 512], dt.float32, tag="ms",
                                  name="msf2")
                    for fc in range(4):
                        nc.tensor.matmul(
                            ps, lhsT=ffw2T_sb[:, fc, :],
                            rhs=hrelu[fc][:, sh * 512:(sh + 1) * 512],
                            start=(fc == 0), stop=(fc == 3))
                    nc.vector.tensor_scalar(
                        ffT[:, sh * 512:(sh + 1) * 512], ps, ffb2_sb,
                        None, ALU.add)
                if KDBG and b == 0:
                    nc.sync.dma_start(out=dbg["ffT"][:], in_=ffT)
                acc = sm.tile([128, 1], dt.float32, tag="acc", name="acc")
                nc.vector.memset(acc, 0.0)
                for sc in range(SC):
                    ps = msp.tile([128, 128], dt.float32, tag="ms",
                                  name="msft")
                    nc.tensor.transpose(ps, ffT[:, sc * 128:(sc + 1) * 128],
                                        ident)
                    l2in = sm.tile([128, 128], dt.float32, tag="l2in",
                                   name="l2in")
                    nc.vector.tensor_tensor(l2in, att[sc], ps, op=ALU.add)
                    l2o = sm.tile([128, 128], dt.float32, tag="l2o",
                                  name="l2o")
                    layer_norm(l2o, l2in, 2)
                    mps = msp.tile([128, 1], dt.float32, tag="ms", name="msm")
                    nc.tensor.matmul(mps, lhsT=l2o, rhs=ones_sb,
                                     start=True, stop=True)
                    nc.vector.tensor_tensor(acc, acc, mps, op=ALU.add)
                ob = sm.tile([128, 1], dt.float32, tag="ob", name="ob")
                nc.scalar.mul(ob, acc, 1.0 / S)
                nc.sync.dma_start(out=yout[b, :, None], in_=ob)

    nc.compile()
    return nc


_CACHE = {}


def kernel(**inputs):
    inputs = {k: np.asarray(v) for k, v in inputs.items()}
    host, index, ln_identity = _host_prep(inputs)
    key = (ln_identity, host["cpack"].shape[1], KDBG, KDBG_G)
    if key not in _CACHE:
        _CACHE[key] = _build_bass(index, host["cpack"].shape[1], ln_identity)
    nc = _CACHE[key]

    from concourse.bass_utils import run_bass_kernel_spmd
    in_maps = _make_in_maps(inputs, host)
    res = run_bass_kernel_spmd(nc, in_maps, list(range(N_CORES)))
    if KDBG:
        kernel.dbg = res.results[0]
    outs = [res.results[c]["y"] for c in range(N_CORES)]
    return np.concatenate(outs, axis=0).astype(F32)


def _make_in_maps(inputs, host):
    x = np.asarray(inputs["x"], dtype=F32)                 # [B, S, 4]
    xpad = np.zeros((B, S + 7, C_IN), F32)
    xpad[:, 3:S + 3, :] = x
    rhs8 = np.empty((B, 8, C_IN, S), F32)
    for t in range(8):
        rhs8[:, t] = xpad[:, t:t + S, :].transpose(0, 2, 1)
    rhs8 = np.ascontiguousarray(rhs8.reshape(B, 8, C_IN * S))
    in_maps = []
    for core in range(N_CORES):
        m = {"rhs8": np.ascontiguousarray(rhs8[core * NB:(core + 1) * NB])}
        m.update(host)
        in_maps.append(m)
    return in_maps


def build(inputs):
    inputs = {k: np.asarray(v) for k, v in inputs.items()}
    host, index, ln_identity = _host_prep(inputs)
    key = (ln_identity, host["cpack"].shape[1], KDBG, KDBG_G)
    if key not in _CACHE:
        _CACHE[key] = _build_bass(index, host["cpack"].shape[1], ln_identity)
    return _CACHE[key], _make_in_maps(inputs, host)
